# revision 1
# baseline (speedup 1.0000x reference)
"""Trainium2 Bass kernel for nn_BaselineProt (embedding_lookup).

The reference computes, per drug-pair sample:
    multihot(drug) @ W0.T  ==  sum of W0 columns at the drug's (deduped)
    target proteins -- i.e. an embedding-table gather/sum, followed by a
    tiny MLP tower on each leg and a dot product between the two legs.

Structure (8 NeuronCores, data-parallel):
  Launch A: drugs sharded 500/core (padded to 512). Each core dma_gathers
      512B bf16 rows of the transposed W0 table for its drugs' targets
      (dups remapped to a zero row so `.set`-style multihot semantics are
      preserved) and tree-reduces them into an E-table shard [512, 256].
  Host:     concatenates the 8 E shards + the 32 cell-line columns into
      one lookup table E_ext [4128, 256] (pure data movement).
  Launch B: batch sharded 1024 samples/core. One transpose-mode gather
      pulls E[d0], E[d1], cell-row per sample in feature-major layout;
      DVE adds + ReLU(+b0) form h0; two matmul layers (W1, W2) and a
      ones-matmul pair-dot produce the [1024] outputs per core.
"""

import os

os.environ.setdefault("JAX_PLATFORMS", "")

import numpy as np
import ml_dtypes

import concourse.bacc as bacc
import concourse.mybir as mybir
from concourse.tile import TileContext
from concourse import library_config
from concourse.bass_utils import run_bass_kernel_spmd

# Problem constants (hardcoded per harness contract).
B = 8192            # samples
P = 19000           # proteins
C = 32              # cell lines
D = 4000            # drugs
T = 32              # targets per drug
F = 256             # first hidden dim
H1 = 128            # second hidden dim
H2 = 64             # output dim per tower

NCORES = 8
DRUGS_PER_CORE = D // NCORES          # 500
DRUGS_PAD = 512                       # per-core padded drug count
SAMPLES_PER_CORE = B // NCORES        # 1024
ZROW = P + C                          # zero row in the W0T table (19032)
TAB_ROWS = ZROW + 8                   # pad table rows to 19040
E_ROWS = NCORES * DRUGS_PAD           # 4096 rows of E
EXT_ROWS = E_ROWS + C                 # + 32 cell rows = 4128
NI_A = DRUGS_PAD * T                  # 16384 gather idxs per core, launch A
NI_B = 3 * SAMPLES_PER_CORE           # 3072 gather idxs per core, launch B
GATHER_SPLIT_A = 32                  # dma_gathers per core in launch A
NQ = 4                                # SWDGE queues

_BF16 = mybir.dt.bfloat16
_F32 = mybir.dt.float32
_I16 = mybir.dt.int16

_cache = {}


def _wrap_idx(flat):
    """Flat gather order -> the [128, n/16] int16 SBUF layout dma_gather
    expects (idx i at partition i%16, slot i//16; replicated to all 8 Q7
    core slices)."""
    n = flat.shape[0]
    assert n % 16 == 0
    arr = flat.astype(np.int16).reshape(n // 16, 16).T.copy()
    return np.tile(arr, (8, 1))


def _build_kernel_a():
    nc = bacc.Bacc("TRN2", target_bir_lowering=True, num_swdge_queues=NQ)
    tab = nc.dram_tensor("tab", [TAB_ROWS, F], _BF16, kind="ExternalInput")
    idxs = nc.dram_tensor("idxs", [128, NI_A // 16], _I16, kind="ExternalInput")
    e_out = nc.dram_tensor("e_out", [DRUGS_PAD, F], _BF16, kind="ExternalOutput")

    ni_s = NI_A // GATHER_SPLIT_A                 # 2048 idxs per gather
    slots_s = ni_s // 128                         # 16 free slots per gather
    n_sub = DRUGS_PAD // 128                      # 4 sub-batches of 128 drugs
    with TileContext(nc) as tc:
        nc.gpsimd.load_library(library_config.mlp)
        with (
            tc.tile_pool(name="idx", bufs=1) as ip,
            tc.tile_pool(name="g", bufs=1) as gp,
            tc.tile_pool(name="e", bufs=2) as ep,
        ):
            idx_t = ip.tile([128, NI_A // 16], _I16)
            nc.sync.dma_start(out=idx_t[:, :], in_=idxs[:, :])
            # issue ALL gathers up front (own tile per sub-batch) so SWDGE
            # generation + drain overlap the DVE reduces end to end
            gs = []
            for b in range(n_sub):
                g = gp.tile([128, T, F], _BF16, tag=f"g{b}")
                nsp = GATHER_SPLIT_A // 4          # gathers per sub-batch
                tsl = T // nsp                     # t-slots per gather
                for h in range(nsp):
                    s = nsp * b + h
                    nc.gpsimd.dma_gather(
                        g[:, h * tsl:(h + 1) * tsl, :],
                        tab[:],
                        idx_t[:, s * (ni_s // 16):(s + 1) * (ni_s // 16)],
                        ni_s, ni_s, F,
                        single_packet=False, queue_num=s % NQ,
                    )
                gs.append(g)
            nsp = GATHER_SPLIT_A // n_sub          # gathers per sub-batch
            tsl = T // nsp                         # t-slots per gather
            for b in range(n_sub):
                g = gs[b]
                # per-gather partial tree (depends on ONE gather's data, so
                # it starts as soon as that gather drains)
                for h in range(nsp):
                    w = tsl // 2
                    while w >= 1:
                        nc.vector.tensor_tensor(
                            out=g[:, h * tsl:h * tsl + w, :],
                            in0=g[:, h * tsl:h * tsl + w, :],
                            in1=g[:, h * tsl + w:h * tsl + 2 * w, :],
                            op=mybir.AluOpType.add,
                        )
                        w //= 2
                # combine the nsp partials (at slots h*tsl) by strided halves
                m = nsp // 2
                while m >= 1:
                    out_ap = g[:, 0:m * tsl:tsl, :]
                    if m == 1:
                        e_strip = ep.tile([128, F], _BF16, tag="e")
                        out_ap = e_strip[:, :].rearrange("p (a f) -> p a f", a=1)
                    nc.vector.tensor_tensor(
                        out=out_ap,
                        in0=g[:, 0:m * tsl:tsl, :],
                        in1=g[:, m * tsl:2 * m * tsl:tsl, :],
                        op=mybir.AluOpType.add,
                    )
                    m //= 2
                nc.sync.dma_start(
                    out=e_out[b * 128:(b + 1) * 128, :], in_=e_strip[:, :]
                )
    nc.compile()
    return nc


def _build_kernel_b():
    nc = bacc.Bacc("TRN2", target_bir_lowering=True, num_swdge_queues=NQ)
    etab = nc.dram_tensor("etab", [EXT_ROWS, F], _BF16, kind="ExternalInput")
    idxs = nc.dram_tensor("idxs", [128, NI_B // 16], _I16, kind="ExternalInput")
    w1t = nc.dram_tensor("w1t", [F, H1], _BF16, kind="ExternalInput")
    w2t = nc.dram_tensor("w2t", [H1, H2], _BF16, kind="ExternalInput")
    b0t = nc.dram_tensor("b0t", [128, 2], _F32, kind="ExternalInput")
    b1t = nc.dram_tensor("b1t", [128, 1], _F32, kind="ExternalInput")
    b2t = nc.dram_tensor("b2t", [64, 1], _F32, kind="ExternalInput")
    y = nc.dram_tensor("y", [1, SAMPLES_PER_CORE], _F32, kind="ExternalOutput")

    S = SAMPLES_PER_CORE                      # 1024
    L = 2 * S                                 # 2048 legs
    NT = 4                                    # matmul N tiles of 512
    TN = L // NT                              # 512
    with TileContext(nc) as tc:
        nc.gpsimd.load_library(library_config.mlp)
        with (
            tc.tile_pool(name="const", bufs=1) as cp,
            tc.tile_pool(name="act", bufs=1) as ap,
            tc.tile_pool(name="ps", bufs=2, space="PSUM") as pp,
        ):
            idx_t = cp.tile([128, NI_B // 16], _I16)
            nc.sync.dma_start(out=idx_t[:, :], in_=idxs[:, :])
            # W1T is [256, H1]; SBUF partition dim is 128 -> [128, 2, H1]
            w1_t = cp.tile([128, 2, H1], _BF16, tag="w1")
            nc.sync.dma_start(
                out=w1_t[:, :, :],
                in_=w1t.ap().rearrange("(c p) h -> p c h", p=128),
            )
            w2_t = cp.tile([128, H2], _BF16, tag="w2")
            nc.sync.dma_start(out=w2_t[:, :], in_=w2t[:, :])
            b0_t = cp.tile([128, 2], _F32, tag="b0")
            nc.sync.dma_start(out=b0_t[:, :], in_=b0t[:, :])
            b1_t = cp.tile([128, 1], _F32, tag="b1")
            nc.sync.dma_start(out=b1_t[:, :], in_=b1t[:, :])
            b2_t = cp.tile([64, 1], _F32, tag="b2")
            nc.sync.dma_start(out=b2_t[:, :], in_=b2t[:, :])
            ones = cp.tile([64, 1], _F32, tag="ones")
            nc.vector.memset(ones[:, :], 1.0)

            # fused gather: per sample s, rows (E[d0], E[d1], cell) at
            # columns 3s, 3s+1, 3s+2; feature-major via transpose mode.
            # 8 gather waves over the 4 queues so early waves' compute
            # starts while later waves still drain. Contiguous dst each.
            NG = 8
            ni_g = NI_B // NG                  # 384 idxs = 128 samples
            sg = ni_g // 3                     # samples per gather tile
            xts = []
            for g in range(NG):
                xt = ap.tile([128, 2, ni_g], _BF16, tag=f"xt{g}")
                nc.gpsimd.dma_gather(
                    xt[:, :, :], etab[:],
                    idx_t[:, g * (ni_g // 16):(g + 1) * (ni_g // 16)],
                    ni_g, ni_g, F,
                    # transpose-mode gathers corrupt (drop a 16-idx slot)
                    # with single_packet=False; they require True
                    transpose=True, single_packet=True, queue_num=g % NQ,
                )
                xts.append(xt)

            # pre-activation: pre[., c, 2s+l] = xt[., c, 3s'+l] + xt[., c, 3s'+2]
            pre = ap.tile([128, 2, L], _F32, tag="pre")
            for g in range(NG):
                for leg in range(2):
                    nc.vector.tensor_tensor(
                        out=pre[:, :, 2 * g * sg + leg:2 * (g + 1) * sg:2],
                        in0=xts[g][:, :, leg:ni_g:3],
                        in1=xts[g][:, :, 2:ni_g:3],
                        op=mybir.AluOpType.add,
                    )
            # h0 = relu(pre + b0), bf16, per feature chunk (bias per partition);
            # split along L per gather wave so matmul tiles pipeline early
            h0 = ap.tile([128, 2, L], _BF16, tag="h0")
            for c in range(2):
                for g in range(NG):
                    nc.scalar.activation(
                        h0[:, c, 2 * g * sg:2 * (g + 1) * sg],
                        pre[:, c, 2 * g * sg:2 * (g + 1) * sg],
                        mybir.ActivationFunctionType.Relu,
                        bias=b0_t[:, c:c + 1], scale=1.0,
                    )

            h1 = ap.tile([128, L], _BF16, tag="h1")
            h2 = ap.tile([64, L], _F32, tag="h2")
            prod = ap.tile([64, S], _F32, tag="prod")
            out_sb = ap.tile([1, S], _F32, tag="out")
            SN = TN // 2                       # 256 samples per tile
            for nt in range(NT):
                ps1 = pp.tile([128, TN], _F32, tag="ps1")
                for c in range(2):
                    nc.tensor.matmul(
                        ps1[:, :], w1_t[:, c, :], h0[:, c, nt * TN:(nt + 1) * TN],
                        start=(c == 0), stop=(c == 1),
                    )
                nc.scalar.activation(
                    h1[:, nt * TN:(nt + 1) * TN], ps1[:, :],
                    mybir.ActivationFunctionType.Relu,
                    bias=b1_t[:, 0:1], scale=1.0,
                )
                ps2 = pp.tile([64, TN], _F32, tag="ps2")
                nc.tensor.matmul(
                    ps2[:, :], w2_t[:, :], h1[:, nt * TN:(nt + 1) * TN],
                    start=True, stop=True,
                )
                nc.scalar.activation(
                    h2[:, nt * TN:(nt + 1) * TN], ps2[:, :],
                    mybir.ActivationFunctionType.Identity,
                    bias=b2_t[:, 0:1], scale=1.0,
                )
                # pair product + dot for this tile's 256 samples
                nc.vector.tensor_tensor(
                    out=prod[:, nt * SN:(nt + 1) * SN],
                    in0=h2[:, nt * TN:(nt + 1) * TN:2],
                    in1=h2[:, nt * TN + 1:(nt + 1) * TN:2],
                    op=mybir.AluOpType.mult,
                )
                ps3 = pp.tile([1, SN], _F32, tag="ps3")
                nc.tensor.matmul(
                    ps3[:, :], ones[:, :], prod[:, nt * SN:(nt + 1) * SN],
                    start=True, stop=True,
                )
                nc.vector.tensor_copy(
                    out_sb[:, nt * SN:(nt + 1) * SN], ps3[:, :]
                )
            nc.sync.dma_start(out=y[:, :], in_=out_sb[:, :])
    nc.compile()
    return nc


def _get_kernels():
    if "a" not in _cache:
        _cache["a"] = _build_kernel_a()
    if "b" not in _cache:
        _cache["b"] = _build_kernel_b()
    return _cache["a"], _cache["b"]


def _prep(drug_pairs, cell_lines, drug_targets, W0, b0, W1, b1, W2, b2):
    """Host-side data layout: shard, transpose, cast, build gather indices."""
    dt = np.asarray(drug_targets, dtype=np.int64)                  # [D, T]
    # dedup per row (reference uses .set -> dup targets count once)
    dup = (dt[:, :, None] == dt[:, None, :]) & (
        np.arange(T)[None, :, None] > np.arange(T)[None, None, :]
    )
    idx = np.where(dup.any(-1), ZROW, dt).astype(np.int32)          # [D, T]

    # W0T table: [P+C rows, F] bf16 + zero row + pad
    w0t = np.zeros((TAB_ROWS, F), dtype=ml_dtypes.bfloat16)
    w0t[: P + C] = np.asarray(W0, np.float32).T.astype(ml_dtypes.bfloat16)

    # launch A per-core gather index arrays
    idx_a = []
    for c in range(NCORES):
        rows = np.full((DRUGS_PAD, T), ZROW, np.int32)
        rows[:DRUGS_PER_CORE] = idx[c * DRUGS_PER_CORE:(c + 1) * DRUGS_PER_CORE]
        # flat j = b*4096 + t*128 + p  ->  drug 128b+p, target t
        flat = rows.reshape(4, 128, T).transpose(0, 2, 1).reshape(-1)
        idx_a.append(_wrap_idx(flat))

    # launch B per-core index arrays (built against E_ext layout)
    dp = np.asarray(drug_pairs, dtype=np.int64)                     # [B, 2]
    cl = np.asarray(cell_lines, dtype=np.int64)                     # [B]
    e_row = (dp // DRUGS_PER_CORE) * DRUGS_PAD + (dp % DRUGS_PER_CORE)
    cell_row = E_ROWS + cl
    idx_b = []
    for c in range(NCORES):
        sl = slice(c * SAMPLES_PER_CORE, (c + 1) * SAMPLES_PER_CORE)
        flat = np.stack(
            [e_row[sl, 0], e_row[sl, 1], cell_row[sl]], axis=1
        ).reshape(-1)
        idx_b.append(_wrap_idx(flat))

    w1t = np.ascontiguousarray(
        np.asarray(W1, np.float32).T.astype(ml_dtypes.bfloat16))    # [F, H1]
    w2t = np.ascontiguousarray(
        np.asarray(W2, np.float32).T.astype(ml_dtypes.bfloat16))    # [H1, H2]
    b0t = np.asarray(b0, np.float32).reshape(2, 128).T.copy()       # [128, 2]
    b1t = np.asarray(b1, np.float32).reshape(128, 1).copy()
    b2t = np.asarray(b2, np.float32).reshape(64, 1).copy()
    celltab = np.asarray(W0, np.float32)[:, P:P + C].T.astype(
        ml_dtypes.bfloat16)                                         # [C, F]
    return w0t, idx_a, idx_b, w1t, w2t, b0t, b1t, b2t, celltab


def _run(inputs, trace=False):
    nca, ncb = _get_kernels()
    w0t, idx_a, idx_b, w1t, w2t, b0t, b1t, b2t, celltab = _prep(**inputs)

    in_a = [{"tab": w0t, "idxs": idx_a[c]} for c in range(NCORES)]
    res_a = run_bass_kernel_spmd(
        nca, in_a, core_ids=list(range(NCORES)), trace=trace)

    e_ext = np.concatenate(
        [res_a.results[c]["e_out"] for c in range(NCORES)] + [celltab], axis=0
    )
    assert e_ext.shape == (EXT_ROWS, F)

    in_b = [
        {"etab": e_ext, "idxs": idx_b[c], "w1t": w1t, "w2t": w2t,
         "b0t": b0t, "b1t": b1t, "b2t": b2t}
        for c in range(NCORES)
    ]
    res_b = run_bass_kernel_spmd(
        ncb, in_b, core_ids=list(range(NCORES)), trace=trace)

    out = np.concatenate(
        [res_b.results[c]["y"].reshape(-1) for c in range(NCORES)]
    ).astype(np.float32)
    times = (res_a.exec_time_ns, res_b.exec_time_ns)
    return out, times


def kernel(**inputs) -> np.ndarray:
    out, _ = _run(inputs, trace=False)
    return out

